# revision 1
# baseline (speedup 1.0000x reference)
"""DeepseekV3 decoder layer (MLA attention + dense MLP) on 8 trn2 NeuronCores.

Strategy: tensor-parallel in transposed-activation space ("T-space").
Activations are stored [feature, token] so every GEMM uses a natural-layout
weight shard as the PE stationary operand and 512-token chunks as the moving
operand (fp32r / bf16 at 1 cycle/row).  All cross-core movement is AllGather
(never AllReduce): each block's final GEMM is column-sharded and the output
is assembled on host from per-core column slices.

Per-core shards (prepared on host in kernel()):
  q_a/kv_a column shards; q_b/kv_b per-head column shards with columns
  reordered (nope|pe resp. k|v) so device rows stay 128-aligned; o/gate/up/
  down column shards.  hidden^T is passed replicated; the core's 512
  residual rows as a separate slice.

Collectives: AG(raw lqT), AG(raw lkvT), AG(attnT bf16), AG(h2T), AG(mT bf16).

All GEMMs run in bf16 (weights host-cast, activations rounded on the
PSUM->SBUF copy); PSUM accumulation, residual adds, softmax exp and norm
statistics stay fp32.  The rope rotation matmul uses an exact +-1 matrix.
"""
import sys

sys.path.insert(0, '/opt/trn_rl_repo')

import numpy as np
import ml_dtypes

S, D, H, QLORA, KVLORA = 1024, 4096, 32, 1536, 512
DN, DR, DV, INTER = 128, 64, 128, 11008
EPS = 1e-6
SCALE = (DN + DR) ** -0.5
NC = 8
HPC = H // NC               # 4 heads per core
QAC = QLORA // NC           # 192 q_a cols per core
KVAC = (KVLORA + DR) // NC  # 72 kv_a cols per core
OC = D // NC                # 512 o_proj/down cols per core
IC = INTER // NC            # 1376 gate/up cols per core

P = 128
TCH = 512                   # moving-operand chunk
NCH = S // TCH              # 2 token chunks
NDT = D // P                # 32
NKVT = KVLORA // P          # 4
NQLT = QLORA // P           # 12
NTT = S // P                # 8
NIT = INTER // P            # 86
NQB = HPC * (DN + DR) // P  # 6 qT row chunks
NOB = OC // P               # 4
BF16 = ml_dtypes.bfloat16

_CACHE = {}


def _build():
    import concourse.bass as bass
    import concourse.tile as tile
    from concourse import bacc, mybir
    from contextlib import ExitStack

    dt = mybir.dt
    f32, f32r, bf16 = dt.float32, dt.float32r, dt.bfloat16
    AF = mybir.ActivationFunctionType
    ts, ds = bass.ts, bass.ds

    nc = bacc.Bacc('TRN2', target_bir_lowering=False, debug=False,
                   num_devices=NC)

    hT = nc.dram_tensor('hT', [D, S], bf16, kind='ExternalInput')
    h_ownD = nc.dram_tensor('h_ownD', [OC, S], f32, kind='ExternalInput')
    qa_own = nc.dram_tensor('qa_own', [D, QAC], bf16, kind='ExternalInput')
    kva_own = nc.dram_tensor('kva_own', [D, KVAC], bf16, kind='ExternalInput')
    qb_own = nc.dram_tensor('qb_own', [QLORA, HPC * (DN + DR)], bf16, kind='ExternalInput')
    kvb_own = nc.dram_tensor('kvb_own', [KVLORA, HPC * (DN + DV)], bf16, kind='ExternalInput')
    o_own = nc.dram_tensor('o_own', [D, OC], bf16, kind='ExternalInput')
    gate_own = nc.dram_tensor('gate_own', [D, IC], bf16, kind='ExternalInput')
    up_own = nc.dram_tensor('up_own', [D, IC], bf16, kind='ExternalInput')
    down_own = nc.dram_tensor('down_own', [INTER, OC], bf16, kind='ExternalInput')
    cosT_d = nc.dram_tensor('cosT2', [P, S], f32, kind='ExternalInput')
    sinT_d = nc.dram_tensor('sinT2', [P, S], f32, kind='ExternalInput')
    rot2_d = nc.dram_tensor('rot2T', [P, P], bf16, kind='ExternalInput')
    masks_d = nc.dram_tensor('masks', [4, P, TCH], bf16, kind='ExternalInput')
    out = nc.dram_tensor('out', [OC, S], f32, kind='ExternalOutput')

    RG = [list(range(NC))]

    def mm(psum, lhsT, rhs, start, stop):
        nc.tensor.matmul(psum, lhsT, rhs, start=start, stop=stop)

    def mmb(psum, lhsT, rhs, start, stop):
        nc.tensor.matmul(psum, lhsT, rhs, start=start, stop=stop)

    with tile.TileContext(nc) as tc, ExitStack() as st:
        const = st.enter_context(tc.tile_pool(name='const', bufs=1))
        vecs = st.enter_context(tc.tile_pool(name='vecs', bufs=1))
        dram = st.enter_context(tc.tile_pool(name='dram', bufs=1, space='DRAM'))

        ones = const.tile([P, 1], f32)
        nc.vector.memset(ones, 1.0)
        ones_bf = const.tile([P, 1], bf16)
        nc.vector.memset(ones_bf, 1.0)
        ones_row = const.tile([1, P], f32)
        nc.vector.memset(ones_row, 1.0)
        eps1 = const.tile([1, 1], f32)
        nc.vector.memset(eps1, EPS)

        def bcast_row(row_ap, name, pool, ps_pool, bufs=1):
            """[1,S] SBUF -> [P,S] SBUF via ones-matmul broadcast."""
            ps = ps_pool.tile([P, S], f32, tag='bc_ps', bufs=1, name=f'{name}_ps')
            for c in range(NCH):
                mm(ps[:, ts(c, TCH)], ones_row, row_ap[0:1, ts(c, TCH)],
                   True, True)
            sb = pool.tile([P, S], f32, tag=f'{name}_bc', bufs=bufs, name=f'{name}_bc')
            nc.vector.tensor_copy(sb, ps)
            return sb

        def finish_norm(ps_sum, scale_meanN, name, extra_sq=None):
            """[1,S] PSUM sumsq -> [1,S] SBUF rsqrt(mean+eps) (optionally *r1^2)."""
            sb = vecs.tile([1, S], f32, tag=f'{name}_v', name=f'{name}_v')
            if extra_sq is not None:
                nc.vector.tensor_mul(sb, ps_sum, extra_sq)
            else:
                nc.vector.tensor_copy(sb, ps_sum)
            nc.scalar.activation(sb, sb, AF.Sqrt, bias=eps1, scale=scale_meanN)
            nc.vector.reciprocal(sb, sb)
            return sb

        lq_dram = dram.tile([QAC, S], bf16)
        lkv_dram = dram.tile([KVAC, S], bf16)
        lq_ag = dram.tile([QLORA, S], bf16, addr_space='Shared')
        lkv_ag = dram.tile([KVLORA + DR, S], bf16, addr_space='Shared')
        attn_dram = dram.tile([HPC * DV, S], bf16)
        attnT_ag = dram.tile([H * DV, S], bf16, addr_space='Shared')
        h2_dram = dram.tile([OC, S], bf16)
        h2_ag = dram.tile([D, S], bf16, addr_space='Shared')
        MA = 768                 # first 6 chunks of m (AG overlaps gate/up tail)
        MB = IC - MA             # 608
        m_dramA = dram.tile([MA, S], bf16)
        m_dramB = dram.tile([MB, S], bf16)
        m_agA = dram.tile([NC * MA, S], bf16, addr_space='Shared')
        m_agB = dram.tile([NC * MB, S], bf16, addr_space='Shared')

        # ============ phase 1: a-projections + input-norm stats ============
        with tc.tile_pool(name='ph1', bufs=3) as ph1, \
             tc.tile_pool(name='ph1ps', bufs=1, space='PSUM') as ph1ps:
            ps_lq = ph1ps.tile([P, S], f32, name='ps_lq')
            ps_lq2 = ph1ps.tile([QAC - P, S], f32, name='ps_lq2')
            ps_lkv = ph1ps.tile([KVAC, S], f32, name='ps_lkv')
            ps_ss1 = ph1ps.tile([1, S], f32, name='ps_ss1')
            G1 = 4
            wkva = ph1.tile([P, NDT, KVAC], bf16, tag='wkva', bufs=1, name='wkva')
            nc.sync.dma_start(out=wkva, in_=kva_own.rearrange('(k p) n -> p k n', p=P))
            for g in range(NDT // G1):
                hk4 = ph1.tile([P, G1, S], bf16, tag='hk4', name='hk4')
                nc.sync.dma_start(
                    out=hk4, in_=hT[g * G1 * P:(g + 1) * G1 * P, :]
                    .rearrange('(k p) s -> p k s', p=P))
                wq4 = ph1.tile([P, G1, QAC], bf16, tag='wq4', name='wq4')
                nc.sync.dma_start(
                    out=wq4, in_=qa_own[g * G1 * P:(g + 1) * G1 * P, :]
                    .rearrange('(k p) n -> p k n', p=P))
                for kk in range(G1):
                    k = g * G1 + kk
                    hk = hk4[:, kk, :]
                    sq = ph1.tile([P, S], bf16, tag='sq', name='sq')
                    nc.vector.tensor_mul(sq, hk, hk)
                    stt, spp = (k == 0), (k == NDT - 1)
                    for c in range(NCH):
                        cs = ts(c, TCH)
                        mm(ps_lq[:, cs], wq4[:, kk, 0:P], hk[:, cs], stt, spp)
                        mm(ps_lq2[:, cs], wq4[:, kk, P:QAC], hk[:, cs], stt, spp)
                        mm(ps_lkv[:, cs], wkva[:, k, :], hk[:, cs], stt, spp)
                        mm(ps_ss1[:, cs], ones_bf[:, 0:1], sq[:, cs], stt, spp)
            r1 = finish_norm(ps_ss1, 1.0 / D, 'r1')
            r1sq = vecs.tile([1, S], f32, name='r1sq')
            nc.vector.tensor_mul(r1sq, r1, r1)
            lq_sb = ph1.tile([P, S], bf16, tag='lq_sb', name='lq_sb')
            nc.vector.tensor_copy(lq_sb, ps_lq)
            nc.sync.dma_start(out=lq_dram[0:P, :], in_=lq_sb)
            lq_sb2 = ph1.tile([QAC - P, S], bf16, tag='lq_sb2', name='lq_sb2')
            nc.vector.tensor_copy(lq_sb2, ps_lq2)
            nc.sync.dma_start(out=lq_dram[P:QAC, :], in_=lq_sb2)
            lkv_sb = ph1.tile([KVAC, S], bf16, tag='lkv_sb', name='lkv_sb')
            nc.vector.tensor_copy(lkv_sb, ps_lkv)
            nc.sync.dma_start(out=lkv_dram[:], in_=lkv_sb)
        nc.gpsimd.collective_compute('AllGather', mybir.AluOpType.bypass,
                                     replica_groups=RG, ins=[lq_dram[:]], outs=[lq_ag[:]])
        nc.gpsimd.collective_compute('AllGather', mybir.AluOpType.bypass,
                                     replica_groups=RG, ins=[lkv_dram[:]], outs=[lkv_ag[:]])

        # pools living through attention
        with ExitStack() as att_st:
            att = att_st.enter_context(tc.tile_pool(name='att', bufs=1))
            qT = att.tile([P, NQB, S], bf16, name='qT')
            kT = att.tile([P, HPC, S], bf16, name='kT')
            v_sb = att.tile([P, NTT, HPC * DV], bf16, name='v_sb')
            kpe = att.tile([P, S], bf16, name='kpe')   # roped k_pe, both halves
            cos_sb = att.tile([P, S], f32, name='cos_sb')
            nc.sync.dma_start(out=cos_sb, in_=cosT_d[:])
            sin_sb = att.tile([P, S], f32, name='sin_sb')
            nc.sync.dma_start(out=sin_sb, in_=sinT_d[:])
            rot2_sb = att.tile([P, P], bf16, name='rot2_sb')
            nc.sync.dma_start(out=rot2_sb, in_=rot2_d[:])
            masks_sb = att.tile([P, 4, TCH], bf16, name='masks_sb')
            nc.sync.dma_start(out=masks_sb, in_=masks_d.rearrange('m p c -> p m c'))

            pre_st = ExitStack()
            pre = pre_st.enter_context(tc.tile_pool(name='pre', bufs=1))
            lqn = pre.tile([P, NQLT, S], bf16, name='lqn')
            kvn = pre.tile([P, NKVT, S], bf16, name='kvn')

            # ============ phase 2: lq/lkv norms, rope k_pe ============
            with tc.tile_pool(name='ph2', bufs=3) as ph2, \
                 tc.tile_pool(name='ph2ps', bufs=1, space='PSUM') as ph2ps:
                ps_ssq = ph2ps.tile([1, S], f32, name='ps_ssq')
                ps_sskv = ph2ps.tile([1, S], f32, name='ps_sskv')
                nc.sync.dma_start(
                    out=lqn, in_=lq_ag.rearrange('(k p) s -> p k s', p=P))
                nc.sync.dma_start(
                    out=kvn, in_=lkv_ag[0:KVLORA, :].rearrange('(k p) s -> p k s', p=P))
                for k in range(NQLT):
                    sq = ph2.tile([P, S], bf16, tag='sq2', bufs=2, name='sq2')
                    nc.vector.tensor_mul(sq, lqn[:, k, :], lqn[:, k, :])
                    for c in range(NCH):
                        mm(ps_ssq[:, ts(c, TCH)], ones_bf[:, 0:1], sq[:, ts(c, TCH)],
                           k == 0, k == NQLT - 1)
                for k in range(NKVT):
                    sq = ph2.tile([P, S], bf16, tag='sq2', bufs=2, name='sq2')
                    nc.vector.tensor_mul(sq, kvn[:, k, :], kvn[:, k, :])
                    for c in range(NCH):
                        mm(ps_sskv[:, ts(c, TCH)], ones_bf[:, 0:1], sq[:, ts(c, TCH)],
                           k == 0, k == NKVT - 1)
                rq = finish_norm(ps_ssq, 1.0 / QLORA, 'rq', extra_sq=r1sq)
                rkv = finish_norm(ps_sskv, 1.0 / KVLORA, 'rkv', extra_sq=r1sq)
                fq = vecs.tile([1, S], f32, name='fq')
                nc.vector.tensor_mul(fq, rq, r1)
                fkv = vecs.tile([1, S], f32, name='fkv')
                nc.vector.tensor_mul(fkv, rkv, r1)
                fq_b = bcast_row(fq, 'fq', ph2, ph2ps)
                fkv_b = bcast_row(fkv, 'fkv', ph2, ph2ps)
                r1_b = bcast_row(r1, 'r1', ph2, ph2ps)
                for k in range(NQLT):
                    nc.vector.tensor_mul(lqn[:, k, :], lqn[:, k, :], fq_b)
                for k in range(NKVT):
                    nc.vector.tensor_mul(kvn[:, k, :], kvn[:, k, :], fkv_b)
                kpe_raw = ph2.tile([DR, S], bf16, tag='kpe_raw', bufs=1, name='kpe_raw')
                nc.sync.dma_start(out=kpe_raw, in_=lkv_ag[KVLORA:KVLORA + DR, :])
                nc.vector.tensor_mul(kpe_raw, kpe_raw, r1_b[0:DR, :])
                # rope: kpe = raw*cos + (R@raw)*sin  (R applied via matmul)
                ps_rot = ph2ps.tile([DR, S], f32, tag='rot_ps', name='rot_ps')
                for c in range(NCH):
                    cs = ts(c, TCH)
                    nc.tensor.matmul(ps_rot[:, cs], rot2_sb[0:DR, 0:DR],
                                     kpe_raw[:, cs], start=True, stop=True)
                rot_s = ph2.tile([DR, S], f32, tag='rot_s', bufs=1, name='rot_s')
                nc.vector.tensor_mul(rot_s, ps_rot, sin_sb[0:DR, :])
                nc.vector.tensor_mul(kpe[0:DR, :], kpe_raw, cos_sb[0:DR, :])
                nc.vector.tensor_add(kpe[0:DR, :], kpe[0:DR, :], rot_s)
                # duplicate into partitions 64:128 (DMA shifts partitions)
                nc.sync.dma_start(out=kpe[DR:P, :], in_=kpe[0:DR, :])

            # ============ phase 3: q_b -> qT ; kv_b -> kT, v ============
            with tc.tile_pool(name='ph3', bufs=3) as ph3, \
                 tc.tile_pool(name='ph3ps', bufs=2, space='PSUM') as ph3ps:
                for mc in range(NQB):
                    ps = ph3ps.tile([P, S], f32, tag='big_ps', name='qT_ps')
                    wq3 = ph3.tile([P, NQLT, P], bf16, tag='wq3', bufs=2, name='wq3')
                    nc.sync.dma_start(
                        out=wq3,
                        in_=qb_own[:, ts(mc, P)].rearrange('(k p) n -> p k n', p=P))
                    for k in range(NQLT):
                        for c in range(NCH):
                            mm(ps[:, ts(c, TCH)], wq3[:, k, :], lqn[:, k, ts(c, TCH)],
                               k == 0, k == NQLT - 1)
                    if mc < HPC * DN // P:
                        nc.vector.tensor_copy(qT[:, mc, :], ps)
                    else:
                        # pe chunk (2 heads x 64 rows): rope via rotation matmul
                        qraw = ph3.tile([P, S], bf16, tag='qraw', bufs=2, name='qraw')
                        nc.vector.tensor_copy(qraw, ps)
                        ps2 = ph3ps.tile([P, S], f32, tag='big_ps', name='rot_q_ps')
                        for c in range(NCH):
                            cs = ts(c, TCH)
                            nc.tensor.matmul(ps2[:, cs], rot2_sb, qraw[:, cs],
                                             start=True, stop=True)
                        rot_s = ph3.tile([P, S], f32, tag='rot_qs', bufs=2, name='rot_qs')
                        nc.vector.tensor_mul(rot_s, ps2, sin_sb)
                        nc.vector.tensor_mul(qT[:, mc, :], qraw, cos_sb)
                        nc.vector.tensor_add(qT[:, mc, :], qT[:, mc, :], rot_s)
                for j in range(HPC):
                    ps = ph3ps.tile([P, S], f32, tag='big_ps', name='kT_ps')
                    wk3 = ph3.tile([P, NKVT, P], bf16, tag='wk3', bufs=2, name='wk3')
                    nc.sync.dma_start(
                        out=wk3,
                        in_=kvb_own[:, ts(j, DN)].rearrange('(k p) n -> p k n', p=P))
                    for k in range(NKVT):
                        for c in range(NCH):
                            mm(ps[:, ts(c, TCH)], wk3[:, k, :], kvn[:, k, ts(c, TCH)],
                               k == 0, k == NKVT - 1)
                    nc.vector.tensor_copy(kT[:, j, :], ps)
                vw = ph3.tile([P, NKVT, HPC * DV], bf16, tag='vw', bufs=1, name='vw')
                nc.sync.dma_start(
                    out=vw, in_=kvb_own[:, HPC * DN:].rearrange('(k p) n -> p k n', p=P))
                for i in range(NTT):
                    ps = ph3ps.tile([P, HPC * DV], f32, tag='v_ps', name='v_ps')
                    for k in range(NKVT):
                        mm(ps, kvn[:, k, ts(i, P)], vw[:, k, :], k == 0, k == NKVT - 1)
                    nc.vector.tensor_copy(v_sb[:, i, :], ps)
            pre_st.close()   # free lqn/kvn before attention

        # ============ phase 4: attention per head ============
            with tc.tile_pool(name='ph4', bufs=2) as ph4, \
                 tc.tile_pool(name='ph4p', bufs=2) as ph4p, \
                 tc.tile_pool(name='ph4ps', bufs=2, space='PSUM') as ph4ps:
                for j in range(HPC):
                    pe_mc = HPC * DN // P + (j * DR) // P
                    pe_off = (j * DR) % P
                    probs = []
                    for i in range(NTT):
                        row = []
                        for jq in range(NCH):
                            if jq < i // 4:
                                row.append(None)
                                continue
                            cs = ts(jq, TCH)
                            ps = ph4ps.tile([P, TCH], f32, tag='sc_ps', bufs=2,
                                            name='sc_ps')
                            mm(ps, kT[:, j, ts(i, P)], qT[:, j, cs], True, False)
                            mm(ps, kpe[pe_off:pe_off + DR, ts(i, P)],
                               qT[pe_off:pe_off + DR, pe_mc, cs], False, True)
                            e = ph4p.tile([P, TCH], bf16, tag=f'probs{i}', bufs=2,
                                          name=f'probs{i}_{jq}')
                            nc.scalar.activation(e, ps, AF.Exp, scale=SCALE)
                            if jq == i // 4:
                                nc.vector.tensor_mul(e, e, masks_sb[:, i % 4, :])
                            row.append(e)
                        probs.append(row)
                    ps_se = ph4ps.tile([1, S], f32, tag='se_ps', bufs=1, name='se_ps')
                    for jq in range(NCH):
                        cs = ts(jq, TCH)
                        valid = [i for i in range(NTT) if jq >= i // 4]
                        for n, i in enumerate(valid):
                            mmb(ps_se[:, cs], ones_bf[:, 0:1], probs[i][jq],
                                n == 0, n == len(valid) - 1)
                    recip = vecs.tile([1, S], f32, tag='recip', name='recip')
                    nc.vector.reciprocal(recip, ps_se)
                    recip_b = bcast_row(recip, 'recip', ph4, ph4ps, bufs=2)
                    for jq in range(NCH):
                        cs = ts(jq, TCH)
                        ps = ph4ps.tile([P, TCH], f32, tag='at_ps', bufs=2, name='at_ps')
                        valid = [i for i in range(NTT) if jq >= i // 4]
                        for n, i in enumerate(valid):
                            mmb(ps, v_sb[:, i, ts(j, DV)], probs[i][jq],
                                n == 0, n == len(valid) - 1)
                        a = ph4.tile([P, TCH], bf16, tag='attn_o', name='attn_o')
                        nc.vector.tensor_mul(a, ps, recip_b[:, cs])
                        nc.sync.dma_start(out=attn_dram[ts(j, DV), cs], in_=a)
        nc.gpsimd.collective_compute('AllGather', mybir.AluOpType.bypass,
                                     replica_groups=RG, ins=[attn_dram[:]], outs=[attnT_ag[:]])

        # ============ phase 5: o_proj + residual ============
        h2own_pool = st.enter_context(tc.tile_pool(name='h2own', bufs=1))
        h2_own_sb = h2own_pool.tile([P, NOB, S], f32, name='h2_own_sb')
        with tc.tile_pool(name='ph5', bufs=3) as ph5, \
             tc.tile_pool(name='ph5r', bufs=1) as ph5r, \
             tc.tile_pool(name='ph5ps', bufs=1, space='PSUM') as ph5ps:
            att_rs = ph5r.tile([P, H * DV // P, S], bf16, name='att_rs')
            nc.sync.dma_start(out=att_rs,
                              in_=attnT_ag.rearrange('(k p) s -> p k s', p=P))
            ps_o = [ph5ps.tile([P, S], f32, tag=f'o_ps{m}', name=f'o_ps{m}')
                    for m in range(NOB)]
            G5 = 8
            for g in range(H * DV // P // G5):
                w8 = ph5.tile([P, G5, OC], bf16, tag='ow8', name='ow8')
                nc.sync.dma_start(
                    out=w8, in_=o_own[g * G5 * P:(g + 1) * G5 * P, :]
                    .rearrange('(k p) n -> p k n', p=P))
                for kk in range(G5):
                    k = g * G5 + kk
                    for mcc in range(NOB):
                        for c in range(NCH):
                            cs = ts(c, TCH)
                            mmb(ps_o[mcc][:, cs], w8[:, kk, ts(mcc, P)],
                                att_rs[:, k, cs], k == 0, k == H * DV // P - 1)
            for mcc in range(NOB):
                hres = ph5.tile([P, S], f32, tag='hres', name='hres')
                nc.sync.dma_start(out=hres, in_=h_ownD[ts(mcc, P), :])
                nc.vector.tensor_add(h2_own_sb[:, mcc, :], ps_o[mcc], hres)
                h2b = ph5.tile([P, S], bf16, tag='h2b', name='h2b')
                nc.vector.tensor_copy(h2b, h2_own_sb[:, mcc, :])
                nc.sync.dma_start(out=h2_dram[ts(mcc, P), :], in_=h2b)
        nc.gpsimd.collective_compute('AllGather', mybir.AluOpType.bypass,
                                     replica_groups=RG, ins=[h2_dram[:]], outs=[h2_ag[:]])

        # ============ phase 6: post-norm + gate/up -> m ============
        with ExitStack() as mlp_st:
            mlp_sb = mlp_st.enter_context(tc.tile_pool(name='mlp_sb', bufs=1))
            h2T = mlp_sb.tile([P, NDT, S], bf16, name='h2T')
            with tc.tile_pool(name='ph6a', bufs=2) as ph6a, \
                 tc.tile_pool(name='ph6aps', bufs=1, space='PSUM') as ph6aps:
                ps_ss2 = ph6aps.tile([1, S], f32, name='ps_ss2')
                nc.sync.dma_start(
                    out=h2T, in_=h2_ag.rearrange('(k p) s -> p k s', p=P))
                for k in range(NDT):
                    sq = ph6a.tile([P, S], bf16, tag='sq6', name='sq6')
                    nc.vector.tensor_mul(sq, h2T[:, k, :], h2T[:, k, :])
                    for c in range(NCH):
                        mm(ps_ss2[:, ts(c, TCH)], ones_bf[:, 0:1], sq[:, ts(c, TCH)],
                           k == 0, k == NDT - 1)
                r2 = finish_norm(ps_ss2, 1.0 / D, 'r2')
                r2_b = bcast_row(r2, 'r2', mlp_sb, ph6aps)

            with tc.tile_pool(name='ph6', bufs=2) as ph6, \
                 tc.tile_pool(name='ph6w', bufs=4) as ph6w, \
                 tc.tile_pool(name='ph6ps', bufs=2, space='PSUM') as ph6ps:
                NMC = (IC + P - 1) // P
                for mcc in range(NMC):
                    rows = min(P, IC - mcc * P)
                    ps_g = ph6ps.tile([P, S], f32, tag='g_ps', name='g_ps')
                    ps_u = ph6ps.tile([P, S], f32, tag='u_ps', name='u_ps')
                    wg = ph6w.tile([P, NDT, rows], bf16, tag='wg', bufs=2, name='wg')
                    nc.sync.dma_start(
                        out=wg, in_=gate_own[:, ds(mcc * P, rows)]
                        .rearrange('(k p) n -> p k n', p=P))
                    wu = ph6w.tile([P, NDT, rows], bf16, tag='wu', bufs=2, name='wu')
                    nc.sync.dma_start(
                        out=wu, in_=up_own[:, ds(mcc * P, rows)]
                        .rearrange('(k p) n -> p k n', p=P))
                    for k in range(NDT):
                        for c in range(NCH):
                            cs = ts(c, TCH)
                            mm(ps_g[0:rows, cs], wg[:, k, :], h2T[:, k, cs],
                               k == 0, k == NDT - 1)
                            mm(ps_u[0:rows, cs], wu[:, k, :], h2T[:, k, cs],
                               k == 0, k == NDT - 1)
                    g = ph6.tile([P, S], f32, tag='g_sb', name='g_sb')
                    nc.vector.tensor_mul(g[0:rows], ps_g[0:rows], r2_b[0:rows])
                    nc.scalar.activation(g[0:rows], g[0:rows], AF.Silu)
                    u = ph6.tile([P, S], f32, tag='u_sb', name='u_sb')
                    nc.vector.tensor_mul(u[0:rows], ps_u[0:rows], r2_b[0:rows])
                    m = ph6.tile([P, S], bf16, tag='m_sb', name='m_sb')
                    nc.vector.tensor_mul(m[0:rows], g[0:rows], u[0:rows])
                    if mcc * P < MA:
                        nc.sync.dma_start(out=m_dramA[ds(mcc * P, rows), :],
                                          in_=m[0:rows])
                    else:
                        nc.sync.dma_start(out=m_dramB[ds(mcc * P - MA, rows), :],
                                          in_=m[0:rows])
        nc.gpsimd.collective_compute('AllGather', mybir.AluOpType.bypass,
                                     replica_groups=RG, ins=[m_dramA[:]], outs=[m_agA[:]])
        nc.gpsimd.collective_compute('AllGather', mybir.AluOpType.bypass,
                                     replica_groups=RG, ins=[m_dramB[:]], outs=[m_agB[:]])

        # ============ phase 7: down_proj + final residual ============
        with tc.tile_pool(name='ph7', bufs=4) as ph7, \
             tc.tile_pool(name='ph7ps', bufs=1, space='PSUM') as ph7ps:
            ps_d = [ph7ps.tile([P, S], f32, tag=f'd_ps{m}', name=f'd_ps{m}')
                    for m in range(NOB)]
            G7 = 2
            NTA = NC * MA // P       # 48 k-tiles in half A
            kglob = 0
            woff = 0
            for src_ag, ntiles in ((m_agA, NC * MA // P), (m_agB, NC * MB // P)):
                for g in range(ntiles // G7):
                    mk = ph7.tile([P, G7, S], bf16, tag='mk', name='mk')
                    nc.sync.dma_start(
                        out=mk, in_=src_ag[g * G7 * P:(g + 1) * G7 * P, :]
                        .rearrange('(k p) s -> p k s', p=P))
                    w = ph7.tile([P, G7, OC], bf16, tag='dw', name='dw')
                    nc.sync.dma_start(
                        out=w, in_=down_own[woff + g * G7 * P:woff + (g + 1) * G7 * P, :]
                        .rearrange('(k p) n -> p k n', p=P))
                    for kk in range(G7):
                        k = kglob + g * G7 + kk
                        for mcc in range(NOB):
                            for c in range(NCH):
                                cs = ts(c, TCH)
                                mmb(ps_d[mcc][:, cs], w[:, kk, ts(mcc, P)],
                                    mk[:, kk, cs], k == 0, k == NIT - 1)
                kglob += ntiles
                woff += ntiles * P
            for mcc in range(NOB):
                o = ph7.tile([P, S], f32, tag='o_out', name='o_out')
                nc.vector.tensor_add(o, ps_d[mcc], h2_own_sb[:, mcc, :])
                nc.sync.dma_start(out=out[ts(mcc, P), :], in_=o)

    nc.compile()
    return nc


def _prep_inputs(inputs):
    """Host-side sharding: returns list of 8 per-core input dicts."""
    h = np.ascontiguousarray(np.asarray(inputs['hidden_states'], np.float32))
    hT = np.ascontiguousarray(h.T)
    cosT = np.ascontiguousarray(np.asarray(inputs['cos'], np.float32).T)
    sinT = np.ascontiguousarray(np.asarray(inputs['sin'], np.float32).T)
    q_a_w = np.asarray(inputs['q_a_w'], np.float32)
    q_b_w = np.asarray(inputs['q_b_w'], np.float32)
    kv_a_w = np.asarray(inputs['kv_a_w'], np.float32)
    kv_b_w = np.asarray(inputs['kv_b_w'], np.float32)
    o_w = np.asarray(inputs['o_w'], np.float32)
    gate_w = np.asarray(inputs['gate_w'], np.float32)
    up_w = np.asarray(inputs['up_w'], np.float32)
    down_w = np.asarray(inputs['down_w'], np.float32)

    pidx = np.arange(P)[:, None]
    cidx = np.arange(TCH)[None, :]
    masks = np.stack([(cidx - pidx >= P * k) for k in range(4)]).astype(BF16)

    # cos/sin duplicated across both 64-partition halves
    cosT2 = np.ascontiguousarray(np.vstack([cosT, cosT]))
    sinT2 = np.ascontiguousarray(np.vstack([sinT, sinT]))
    # rotation matrix: rot(x) = R @ x with R[m, m+32] = -1, R[m+32, m] = +1
    # (per 64-row head block, two blocks stacked).  Passed as R2.T = lhsT.
    R = np.zeros((DR, DR), np.float32)
    R[np.arange(DR // 2), np.arange(DR // 2) + DR // 2] = -1.0
    R[np.arange(DR // 2) + DR // 2, np.arange(DR // 2)] = 1.0
    R2 = np.zeros((P, P), np.float32)
    R2[:DR, :DR] = R
    R2[DR:, DR:] = R
    rot2T = np.ascontiguousarray(R2.T)

    # down rows reordered to match the two-part m AllGather layout:
    # [rank-major rows 0:768 of each core's shard, then rows 768:1376]
    MA = 768
    m_row_order = np.concatenate(
        [np.arange(MA) + rr * IC for rr in range(NC)] +
        [np.arange(MA, IC) + rr * IC for rr in range(NC)])

    in_maps = []
    for r in range(NC):
        heads = range(r * HPC, (r + 1) * HPC)
        qb_cols = np.concatenate(
            [q_b_w[:, hh * (DN + DR):hh * (DN + DR) + DN] for hh in heads] +
            [q_b_w[:, hh * (DN + DR) + DN:(hh + 1) * (DN + DR)] for hh in heads],
            axis=1)
        kvb_cols = np.concatenate(
            [kv_b_w[:, hh * (DN + DV):hh * (DN + DV) + DN] for hh in heads] +
            [kv_b_w[:, hh * (DN + DV) + DN:(hh + 1) * (DN + DV)] for hh in heads],
            axis=1)
        in_maps.append({
            'hT': hT.astype(BF16),
            'h_ownD': np.ascontiguousarray(hT[r * OC:(r + 1) * OC]),
            'qa_own': np.ascontiguousarray(q_a_w[:, r * QAC:(r + 1) * QAC]).astype(BF16),
            'kva_own': np.ascontiguousarray(kv_a_w[:, r * KVAC:(r + 1) * KVAC]).astype(BF16),
            'qb_own': np.ascontiguousarray(qb_cols).astype(BF16),
            'kvb_own': np.ascontiguousarray(kvb_cols).astype(BF16),
            'o_own': np.ascontiguousarray(o_w[:, r * OC:(r + 1) * OC]).astype(BF16),
            'gate_own': np.ascontiguousarray(gate_w[:, r * IC:(r + 1) * IC]).astype(BF16),
            'up_own': np.ascontiguousarray(up_w[:, r * IC:(r + 1) * IC]).astype(BF16),
            'down_own': np.ascontiguousarray(
                down_w[m_row_order, r * OC:(r + 1) * OC]).astype(BF16),
            'cosT2': cosT2,
            'sinT2': sinT2,
            'rot2T': rot2T.astype(BF16),
            'masks': masks,
        })
    return in_maps


def kernel(**inputs) -> np.ndarray:
    if 'nc' not in _CACHE:
        _CACHE['nc'] = _build()
    nc = _CACHE['nc']
    from concourse.bass_utils import run_bass_kernel_spmd
    in_maps = _prep_inputs(inputs)
    res = run_bass_kernel_spmd(nc, in_maps, core_ids=list(range(NC)))
    outT = np.concatenate([res.results[r]['out'] for r in range(NC)], axis=0)
    return np.ascontiguousarray(outT.T)



# revision 16
# speedup vs baseline: 1.2126x; 1.2126x over previous
"""DeepseekV3 decoder layer (MLA attention + dense MLP) on 8 trn2 NeuronCores.

Tensor-parallel in transposed-activation space ("T-space"): activations are
stored [feature, token] so every GEMM uses a natural-layout weight shard as
the PE stationary operand and 512-token chunks as the bf16 moving operand.
All cross-core movement is AllGather of column-sharded block outputs.

Pipelining strategy (vs the phase-serial baseline):
  * phase 1 computes the kv_a columns first so AG(lkv) overlaps the q_a
    tail; AG(lq) overlaps the kv_b-side work (kT / v / k_pe rope).
  * per-token rmsnorm statistics ride the AllGathers as extra bf16 rows
    (per-core partial sum-of-squares), then one tiny reduce matmul + a
    ones-broadcast matmul rebuilds the full [128,S] scale tile; rsqrt is
    ACT Sqrt + DVE reciprocal_approx_fast.  No [128,S] DVE squares over
    gathered tensors, no fp32 moving operands.
  * attention output is AllGathered per head (4 waves); o_proj consumes
    k-rows wave-by-wave (o_w host-reordered) accumulating in SBUF fp32,
    so o_proj overlaps the attention AGs.
  * h2 (= residual + o) is AllGathered per 128-row chunk as it drains,
    with its norm partial riding each chunk; gate/up consume k-tiles in
    chunk-wave order (gate/up weights host-reordered), and the r2 scale
    is only needed at the first PSUM drain, ~27us after the MMs start.
  * m is AllGathered in 3 groups so down_proj overlaps the gate/up tail.
"""
import sys

sys.path.insert(0, '/opt/trn_rl_repo')

import numpy as np
import ml_dtypes

S, D, H, QLORA, KVLORA = 1024, 4096, 32, 1536, 512
DN, DR, DV, INTER = 128, 64, 128, 11008
EPS = 1e-6
SCALE = (DN + DR) ** -0.5
NC = 8
HPC = H // NC               # 4 heads per core
QAC = QLORA // NC           # 192 q_a cols per core
KVAC = (KVLORA + DR) // NC  # 72 kv_a cols per core
OC = D // NC                # 512 o_proj/down cols per core
IC = INTER // NC            # 1376 gate/up cols per core

P = 128
TCH = 512                   # moving-operand chunk
NCH = S // TCH              # 2 token chunks
NDT = D // P                # 32
NKVT = KVLORA // P          # 4
NQLT = QLORA // P           # 12
NTT = S // P                # 8
NQB = HPC * (DN + DR) // P  # 6 qT row chunks
APAD = 24                   # zero pad so q cols start at partition 96
AWC = KVAC + APAD + QAC     # 288 phase-1 weight cols (kv | pad | q)
LKB = KVAC + 2              # 74: lkv AG block rows (72 vals + ss_in + ss_kv)
LQB = QAC + 1               # 193: lq AG block rows (192 vals + ss_lq)
H2B = P + 1                 # 129: h2 AG block rows per chunk
NMC = (IC + P - 1) // P     # 11 gate/up row chunks (last is 96)
MGRP = [(0, 4, 512), (4, 8, 512), (8, 11, 352)]   # m AG groups (mc lo, hi, rows)
BF16 = ml_dtypes.bfloat16

_CACHE = {}


def _pieces(nblk, rows_of, stride, f0_of):
    """DMA piece list mapping AG blocks to [128, k, S] SBUF tiles."""
    out = []
    for c in range(nblk):
        j, rows, f0 = 0, rows_of(c), f0_of(c)
        while j < rows:
            f = f0 + j
            k, p = f // P, f % P
            take = min(rows - j, P - p)
            out.append((c * stride + j, take, k, p))
            j += take
    return out


def _build():
    import concourse.bass as bass
    import concourse.tile as tile
    from concourse import bacc, mybir
    from contextlib import ExitStack

    dt = mybir.dt
    f32, bf16 = dt.float32, dt.bfloat16
    AF = mybir.ActivationFunctionType
    ts, ds = bass.ts, bass.ds

    nc = bacc.Bacc('TRN2', target_bir_lowering=False, debug=False,
                   num_devices=NC)

    hT = nc.dram_tensor('hT', [D, S], bf16, kind='ExternalInput')
    h_ownD = nc.dram_tensor('h_ownD', [OC, S], f32, kind='ExternalInput')
    aw = nc.dram_tensor('aw', [D, AWC], bf16, kind='ExternalInput')
    kvmask_d = nc.dram_tensor('kvmask', [KVAC, 1], bf16, kind='ExternalInput')
    qb_own = nc.dram_tensor('qb_own', [QLORA, HPC * (DN + DR)], bf16, kind='ExternalInput')
    kvb_own = nc.dram_tensor('kvb_own', [KVLORA, HPC * (DN + DV)], bf16, kind='ExternalInput')
    o_own = nc.dram_tensor('o_own', [D, OC], bf16, kind='ExternalInput')
    gate_own = nc.dram_tensor('gate_own', [D, IC], bf16, kind='ExternalInput')
    up_own = nc.dram_tensor('up_own', [D, IC], bf16, kind='ExternalInput')
    down_own = nc.dram_tensor('down_own', [INTER, OC], bf16, kind='ExternalInput')
    cosT_d = nc.dram_tensor('cosT2', [P, S], f32, kind='ExternalInput')
    sinT_d = nc.dram_tensor('sinT2', [P, S], f32, kind='ExternalInput')
    rot2_d = nc.dram_tensor('rot2T', [P, P], bf16, kind='ExternalInput')
    masks_d = nc.dram_tensor('masks', [4, P, TCH], bf16, kind='ExternalInput')
    out = nc.dram_tensor('out', [OC, S], f32, kind='ExternalOutput')

    RG = [list(range(NC))]

    def mm(psum, lhsT, rhs, start, stop):
        nc.tensor.matmul(psum, lhsT, rhs, start=start, stop=stop)

    def ag(src, dst):
        nc.gpsimd.collective_compute('AllGather', mybir.AluOpType.bypass,
                                     replica_groups=RG, ins=[src[:]], outs=[dst[:]])

    with tile.TileContext(nc) as tc, ExitStack() as st:
        const = st.enter_context(tc.tile_pool(name='const', bufs=1))
        vecs = st.enter_context(tc.tile_pool(name='vecs', bufs=1))
        dram = st.enter_context(tc.tile_pool(name='dram', bufs=1, space='DRAM'))

        ones1_128 = const.tile([1, P], bf16)
        nc.vector.memset(ones1_128, 1.0)
        ones8_1 = const.tile([8, 1], bf16)
        nc.vector.memset(ones8_1, 1.0)
        ones32_1 = const.tile([32, 1], bf16)
        nc.vector.memset(ones32_1, 1.0)
        ones128_1 = const.tile([P, 1], bf16)
        nc.vector.memset(ones128_1, 1.0)
        eps1 = const.tile([1, 1], f32)
        nc.vector.memset(eps1, EPS)

        def row_chain(ps_row, meanN, name, pool):
            """[1,S] PSUM sumsq -> ([1,S] bf16 rsqrt row, f32 row)."""
            sq = vecs.tile([1, S], f32, tag=f'{name}_sq', name=f'{name}_sq')
            nc.scalar.activation(sq, ps_row, AF.Sqrt, bias=eps1, scale=meanN)
            rec = vecs.tile([1, S], f32, tag=f'{name}_rc', name=f'{name}_rc')
            nc.vector.reciprocal_approx_fast(out=rec, in_=sq)
            row = vecs.tile([1, S], bf16, tag=f'{name}_row', name=f'{name}_row')
            nc.vector.tensor_copy(row, rec)
            return row

        def bcast(row_bf, ps_pool, pool, name, dtype=bf16, tag=None):
            """[1,S] bf16 row -> [128,S] SBUF via ones-matmul broadcast."""
            ps = ps_pool.tile([P, S], f32, tag=tag or f'{name}_bps', bufs=1,
                             name=f'{name}_bps')
            for c in range(NCH):
                mm(ps[:, ts(c, TCH)], ones1_128, row_bf[0:1, ts(c, TCH)],
                   True, True)
            sb = pool.tile([P, S], dtype, tag=f'{name}_b', bufs=1, name=f'{name}_b')
            nc.vector.tensor_copy(sb, ps)
            return sb

        lkv_dram = dram.tile([LKB, S], bf16, name='lkv_dram')
        lq_dram = dram.tile([LQB, S], bf16, name='lq_dram')
        lkv_ag = dram.tile([NC * LKB, S], bf16, addr_space='Shared', name='lkv_ag')
        lq_ag = dram.tile([NC * LQB, S], bf16, addr_space='Shared', name='lq_ag')
        attn_dram = [dram.tile([DV, S], bf16, name=f'attn_dram{j}') for j in range(HPC)]
        attn_ag = [dram.tile([NC * DV, S], bf16, addr_space='Shared',
                             name=f'attn_ag{j}') for j in range(HPC)]
        h2_dram = [dram.tile([H2B, S], bf16, name=f'h2_dram{j}') for j in range(4)]
        h2_ag = [dram.tile([NC * H2B, S], bf16, addr_space='Shared',
                           name=f'h2_ag{j}') for j in range(4)]
        m_dram = [dram.tile([g[2], S], bf16, name=f'm_dram{i}') for i, g in enumerate(MGRP)]
        m_ag = [dram.tile([NC * g[2], S], bf16, addr_space='Shared',
                          name=f'm_ag{i}') for i, g in enumerate(MGRP)]

        # persistent across phases
        h_own = st.enter_context(tc.tile_pool(name='hown', bufs=1)) \
            .tile([P, OC // P, S], f32, name='h_own')
        nc.sync.dma_start(out=h_own, in_=h_ownD.rearrange('(k p) s -> p k s', p=P))
        o_acc_pool = st.enter_context(tc.tile_pool(name='oacc', bufs=1))
        o_acc = o_acc_pool.tile([P, OC // P, S], f32, name='o_acc')

        # ============ phase 1: a-projections + ride-along norm partials ====
        with tc.tile_pool(name='ph1', bufs=2) as ph1, \
             tc.tile_pool(name='ph1h', bufs=1) as ph1h, \
             tc.tile_pool(name='ph1ps', bufs=1, space='PSUM') as ph1ps:
            kvmask_sb = ph1h.tile([KVAC, 1], bf16, name='kvmask_sb')
            nc.sync.dma_start(out=kvmask_sb, in_=kvmask_d[:])
            hk = ph1h.tile([P, NDT, S], bf16, name='hk')
            G1 = 4
            for g in range(NDT // G1):
                nc.sync.dma_start(
                    out=hk[:, g * G1:(g + 1) * G1, :],
                    in_=hT[g * G1 * P:(g + 1) * G1 * P, :]
                    .rearrange('(k p) s -> p k s', p=P))
            w0 = ph1h.tile([P, NDT, P], bf16, name='w0')
            nc.sync.dma_start(out=w0, in_=aw[:, 0:P].rearrange('(k p) n -> p k n', p=P))
            w1 = ph1h.tile([P, NDT, P], bf16, name='w1')
            nc.sync.dma_start(out=w1, in_=aw[:, P:2 * P].rearrange('(k p) n -> p k n', p=P))
            w2 = ph1h.tile([P, NDT, AWC - 2 * P], bf16, name='w2')
            nc.sync.dma_start(out=w2, in_=aw[:, 2 * P:AWC].rearrange('(k p) n -> p k n', p=P))

            pc0 = ph1ps.tile([P, S], f32, name='pc0')
            pc1 = ph1ps.tile([P, S], f32, name='pc1')
            pc2 = ph1ps.tile([AWC - 2 * P, S], f32, name='pc2')
            pp = ph1ps.tile([1, S], f32, tag='pp', bufs=1, name='pp')

            # input-norm partial over own OC rows (from h_own, fp32)
            for t in range(OC // P):
                sqh = ph1.tile([P, S], bf16, tag='sqh', name='sqh')
                nc.vector.tensor_mul(sqh, h_own[:, t, :], h_own[:, t, :])
                for c in range(NCH):
                    mm(pp[0:1, ts(c, TCH)], ones128_1, sqh[:, ts(c, TCH)],
                       t == 0, t == OC // P - 1)
            ip_row = ph1.tile([1, S], bf16, tag='prow', name='ip_row')
            nc.scalar.activation(ip_row, pp, AF.Copy)
            nc.sync.dma_start(out=lkv_dram[KVAC:KVAC + 1, :], in_=ip_row)

            # chunk 0 first (contains all kv_a cols) so AG(lkv) fires early
            for k in range(NDT):
                for c in range(NCH):
                    mm(pc0[:, ts(c, TCH)], w0[:, k, :], hk[:, k, ts(c, TCH)],
                       k == 0, k == NDT - 1)
            lkv_sb = ph1.tile([KVAC, S], bf16, tag='lkv_sb', bufs=1, name='lkv_sb')
            nc.vector.tensor_copy(lkv_sb, pc0[0:KVAC, :])
            nc.sync.dma_start(out=lkv_dram[0:KVAC, :], in_=lkv_sb)
            lq0 = ph1.tile([P - KVAC - APAD, S], bf16, tag='lq0', bufs=1, name='lq0')
            nc.vector.tensor_copy(lq0, pc0[KVAC + APAD:P, :])
            nc.sync.dma_start(out=lq_dram[0:P - KVAC - APAD, :], in_=lq0)
            # kv-norm partial (masked to kvlora columns)
            sq0 = ph1.tile([KVAC, S], bf16, tag='sq0', bufs=1, name='sq0')
            nc.scalar.activation(sq0, pc0[0:KVAC, :], AF.Square)
            pp_kv = ph1ps.tile([1, S], f32, tag='pp', name='pp_kv')
            for c in range(NCH):
                mm(pp_kv[0:1, ts(c, TCH)], kvmask_sb, sq0[:, ts(c, TCH)],
                   True, True)
            kv_row = ph1.tile([1, S], bf16, tag='prow', name='kv_row')
            nc.scalar.activation(kv_row, pp_kv, AF.Copy)
            nc.sync.dma_start(out=lkv_dram[KVAC + 1:KVAC + 2, :], in_=kv_row)
            ag(lkv_dram, lkv_ag)

            # chunks 1-2 (rest of q_a)
            for k in range(NDT):
                for c in range(NCH):
                    mm(pc1[:, ts(c, TCH)], w1[:, k, :], hk[:, k, ts(c, TCH)],
                       k == 0, k == NDT - 1)
            lq1 = ph1.tile([P, S], bf16, tag='lq1', bufs=1, name='lq1')
            nc.vector.tensor_copy(lq1, pc1)
            nc.sync.dma_start(out=lq_dram[P - KVAC - APAD:2 * P - KVAC - APAD, :], in_=lq1)
            for k in range(NDT):
                for c in range(NCH):
                    mm(pc2[:, ts(c, TCH)], w2[:, k, :], hk[:, k, ts(c, TCH)],
                       k == 0, k == NDT - 1)
            lq2 = ph1.tile([AWC - 2 * P, S], bf16, tag='lq2', bufs=1, name='lq2')
            nc.vector.tensor_copy(lq2, pc2)
            nc.sync.dma_start(out=lq_dram[2 * P - KVAC - APAD:QAC, :], in_=lq2)
            # q-norm partial over own QAC cols (from fp32 psum)
            sqq0 = ph1.tile([P - KVAC - APAD, S], bf16, tag='sqq0', bufs=1, name='sqq0')
            nc.scalar.activation(sqq0, pc0[KVAC + APAD:P, :], AF.Square)
            sqq1 = ph1.tile([P, S], bf16, tag='sqq1', bufs=1, name='sqq1')
            nc.scalar.activation(sqq1, pc1, AF.Square)
            sqq2 = ph1.tile([AWC - 2 * P, S], bf16, tag='sqq2', bufs=1, name='sqq2')
            nc.scalar.activation(sqq2, pc2, AF.Square)
            pp_lq = ph1ps.tile([1, S], f32, tag='pp', name='pp_lq')
            for c in range(NCH):
                cs = ts(c, TCH)
                mm(pp_lq[0:1, cs], ones128_1[0:P - KVAC - APAD, :], sqq0[:, cs], True, False)
                mm(pp_lq[0:1, cs], ones128_1[0:P, :], sqq1[:, cs], False, False)
                mm(pp_lq[0:1, cs], ones128_1[0:AWC - 2 * P, :], sqq2[:, cs], False, True)
            lq_row = ph1.tile([1, S], bf16, tag='prow', name='lq_row')
            nc.scalar.activation(lq_row, pp_lq, AF.Copy)
            nc.sync.dma_start(out=lq_dram[QAC:QAC + 1, :], in_=lq_row)
            ag(lq_dram, lq_ag)

        # pools living through attention
        with ExitStack() as att_st:
            att = att_st.enter_context(tc.tile_pool(name='att', bufs=1))
            qT = att.tile([P, NQB, S], bf16, name='qT')
            kT = att.tile([P, HPC, S], bf16, name='kT')
            v_sb = att.tile([P, NTT, HPC * DV], bf16, name='v_sb')
            kpe = att.tile([P, S], bf16, name='kpe')
            masks_sb = att.tile([P, 4, TCH], bf16, name='masks_sb')
            nc.sync.dma_start(out=masks_sb, in_=masks_d.rearrange('m p c -> p m c'))

            pre_st = ExitStack()
            pre = pre_st.enter_context(tc.tile_pool(name='pre', bufs=1))
            lqn = pre.tile([P, NQLT, S], bf16, name='lqn')
            kvn = pre.tile([P, NKVT, S], bf16, name='kvn')
            cos_sb = pre.tile([P, S], f32, name='cos_sb')
            nc.sync.dma_start(out=cos_sb, in_=cosT_d[:])
            sin_sb = pre.tile([P, S], f32, name='sin_sb')
            nc.sync.dma_start(out=sin_sb, in_=sinT_d[:])
            rot2_sb = pre.tile([P, P], bf16, name='rot2_sb')
            nc.sync.dma_start(out=rot2_sb, in_=rot2_d[:])

            # ---- kv-side: norms, kpe rope, kT, v (overlaps AG(lq)) ----
            with tc.tile_pool(name='ph2', bufs=2) as ph2, \
                 tc.tile_pool(name='ph2ps', bufs=1, space='PSUM') as ph2ps:
                for (r0, take, k, p) in _pieces(
                        NC, lambda c: KVAC if c < 7 else KVLORA - 7 * KVAC,
                        LKB, lambda c: c * KVAC):
                    nc.sync.dma_start(out=kvn[p:p + take, k, :],
                                      in_=lkv_ag[r0:r0 + take, :])
                ippart = ph2.tile([8, S], bf16, tag='ippart', bufs=1, name='ippart')
                kvpart = ph2.tile([8, S], bf16, tag='kvpart', bufs=1, name='kvpart')
                for c in range(NC):
                    nc.sync.dma_start(out=ippart[c:c + 1, :],
                                      in_=lkv_ag[c * LKB + KVAC:c * LKB + KVAC + 1, :])
                    nc.sync.dma_start(out=kvpart[c:c + 1, :],
                                      in_=lkv_ag[c * LKB + KVAC + 1:c * LKB + KVAC + 2, :])
                pr1 = ph2ps.tile([1, S], f32, tag='prow_ps', bufs=2, name='pr1')
                for c in range(NCH):
                    mm(pr1[0:1, ts(c, TCH)], ones8_1, ippart[:, ts(c, TCH)], True, True)
                r1_row = row_chain(pr1, 1.0 / D, 'r1', ph2)
                r1_b = bcast(r1_row, ph2ps, ph2, 'r1', tag='bps')
                pkv = ph2ps.tile([1, S], f32, tag='prow_ps', bufs=2, name='pkv')
                for c in range(NCH):
                    mm(pkv[0:1, ts(c, TCH)], ones8_1, kvpart[:, ts(c, TCH)], True, True)
                fkv_row = row_chain(pkv, 1.0 / KVLORA, 'fkv', ph2)
                fkv_b = bcast(fkv_row, ph2ps, ph2, 'fkv', tag='bps')
                for k in range(NKVT):
                    nc.vector.tensor_mul(kvn[:, k, :], kvn[:, k, :], fkv_b)
                # k_pe: scale by r1, rope via rotation matmul
                kpe_raw = ph2.tile([DR, S], bf16, tag='kpe_raw', bufs=1, name='kpe_raw')
                nc.sync.dma_start(out=kpe_raw,
                                  in_=lkv_ag[7 * LKB + (KVLORA - 7 * KVAC):7 * LKB + KVAC, :])
                nc.vector.tensor_mul(kpe_raw, kpe_raw, r1_b[0:DR, :])
                ps_rot = ph2ps.tile([DR, S], f32, tag='rot_ps', name='rot_ps')
                for c in range(NCH):
                    cs = ts(c, TCH)
                    nc.tensor.matmul(ps_rot[:, cs], rot2_sb[0:DR, 0:DR],
                                     kpe_raw[:, cs], start=True, stop=True)
                rot_s = ph2.tile([DR, S], f32, tag='rot_s', bufs=1, name='rot_s')
                nc.vector.tensor_mul(rot_s, ps_rot, sin_sb[0:DR, :])
                nc.vector.tensor_mul(kpe[0:DR, :], kpe_raw, cos_sb[0:DR, :])
                nc.vector.tensor_add(kpe[0:DR, :], kpe[0:DR, :], rot_s)
                nc.sync.dma_start(out=kpe[DR:P, :], in_=kpe[0:DR, :])

            with tc.tile_pool(name='ph3k', bufs=2) as ph3k, \
                 tc.tile_pool(name='ph3kps', bufs=2, space='PSUM') as ph3kps:
                for j in range(HPC):
                    ps = ph3kps.tile([P, S], f32, tag='kt_ps', name='kt_ps')
                    wk3 = ph3k.tile([P, NKVT, P], bf16, tag='wk3', bufs=2, name='wk3')
                    nc.sync.dma_start(
                        out=wk3,
                        in_=kvb_own[:, ts(j, DN)].rearrange('(k p) n -> p k n', p=P))
                    for k in range(NKVT):
                        for c in range(NCH):
                            mm(ps[:, ts(c, TCH)], wk3[:, k, :], kvn[:, k, ts(c, TCH)],
                               k == 0, k == NKVT - 1)
                    nc.vector.tensor_copy(kT[:, j, :], ps)
                vw = ph3k.tile([P, NKVT, HPC * DV], bf16, tag='vw', bufs=1, name='vw')
                nc.sync.dma_start(
                    out=vw, in_=kvb_own[:, HPC * DN:].rearrange('(k p) n -> p k n', p=P))
                for i in range(NTT):
                    ps = ph3kps.tile([P, HPC * DV], f32, tag='v_ps', name='v_ps')
                    for k in range(NKVT):
                        mm(ps, kvn[:, k, ts(i, P)], vw[:, k, :], k == 0, k == NKVT - 1)
                    nc.vector.tensor_copy(v_sb[:, i, :], ps)

            # ---- q-side: fq, qT (after AG(lq)) ----
            with tc.tile_pool(name='ph3q', bufs=2) as ph3q, \
                 tc.tile_pool(name='ph3qps', bufs=1, space='PSUM') as ph3qps:
                for (r0, take, k, p) in _pieces(NC, lambda c: QAC, LQB,
                                                lambda c: c * QAC):
                    nc.sync.dma_start(out=lqn[p:p + take, k, :],
                                      in_=lq_ag[r0:r0 + take, :])
                lqpart = ph3q.tile([8, S], bf16, tag='lqpart', bufs=1, name='lqpart')
                for c in range(NC):
                    nc.sync.dma_start(out=lqpart[c:c + 1, :],
                                      in_=lq_ag[c * LQB + QAC:c * LQB + QAC + 1, :])
                plq = ph3qps.tile([1, S], f32, tag='plq', bufs=1, name='plq')
                for c in range(NCH):
                    mm(plq[0:1, ts(c, TCH)], ones8_1, lqpart[:, ts(c, TCH)], True, True)
                fq_row = row_chain(plq, 1.0 / QLORA, 'fq', ph3q)
                fq_b = bcast(fq_row, ph3qps, att, 'fq', dtype=f32)

            with tc.tile_pool(name='ph3', bufs=2) as ph3, \
                 tc.tile_pool(name='ph3ps', bufs=2, space='PSUM') as ph3ps:
                for mc in range(NQB):
                    ps = ph3ps.tile([P, S], f32, tag='qT_ps', name='qT_ps')
                    wq3 = ph3.tile([P, NQLT, P], bf16, tag='wq3', bufs=2, name='wq3')
                    nc.sync.dma_start(
                        out=wq3,
                        in_=qb_own[:, ts(mc, P)].rearrange('(k p) n -> p k n', p=P))
                    for k in range(NQLT):
                        for c in range(NCH):
                            mm(ps[:, ts(c, TCH)], wq3[:, k, :], lqn[:, k, ts(c, TCH)],
                               k == 0, k == NQLT - 1)
                    if mc < HPC * DN // P:
                        nc.vector.tensor_mul(qT[:, mc, :], ps, fq_b)
                    else:
                        qraw = ph3.tile([P, S], bf16, tag='qraw', bufs=2, name='qraw')
                        nc.vector.tensor_mul(qraw, ps, fq_b)
                        ps2 = ph3ps.tile([P, S], f32, tag='qT_ps', name='rot_q_ps')
                        for c in range(NCH):
                            cs = ts(c, TCH)
                            nc.tensor.matmul(ps2[:, cs], rot2_sb, qraw[:, cs],
                                             start=True, stop=True)
                        rot_s = ph3.tile([P, S], f32, tag='rot_qs', bufs=2, name='rot_qs')
                        nc.vector.tensor_mul(rot_s, ps2, sin_sb)
                        nc.vector.tensor_mul(qT[:, mc, :], qraw, cos_sb)
                        nc.vector.tensor_add(qT[:, mc, :], qT[:, mc, :], rot_s)
            pre_st.close()   # free lqn/kvn before attention

        # ============ attention + per-head AG + wave-pipelined o_proj ======
            with tc.tile_pool(name='ph4', bufs=2) as ph4, \
                 tc.tile_pool(name='ph4p', bufs=2) as ph4p, \
                 tc.tile_pool(name='ph4w', bufs=1) as ph4w, \
                 tc.tile_pool(name='ph45ps', bufs=1, space='PSUM') as ph45ps:

                for j in range(HPC):
                    pe_mc = HPC * DN // P + (j * DR) // P
                    pe_off = (j * DR) % P
                    probs = []
                    for i in range(NTT):
                        row = []
                        for jq in range(NCH):
                            if jq < i // 4:
                                row.append(None)
                                continue
                            cs = ts(jq, TCH)
                            ps = ph45ps.tile([P, TCH], f32, tag='sc_ps', bufs=2,
                                             name='sc_ps')
                            mm(ps, kT[:, j, ts(i, P)], qT[:, j, cs], True, False)
                            mm(ps, kpe[pe_off:pe_off + DR, ts(i, P)],
                               qT[pe_off:pe_off + DR, pe_mc, cs], False, True)
                            e = ph4p.tile([P, TCH], bf16, tag=f'probs{i}', bufs=2,
                                          name=f'probs{i}_{jq}')
                            nc.scalar.activation(e, ps, AF.Exp, scale=SCALE)
                            if jq == i // 4:
                                nc.vector.tensor_mul(e, e, masks_sb[:, i % 4, :])
                            row.append(e)
                        probs.append(row)
                    ps_se = ph45ps.tile([1, S], f32, tag='se_ps', bufs=1, name='se_ps')
                    for jq in range(NCH):
                        cs = ts(jq, TCH)
                        valid = [i for i in range(NTT) if jq >= i // 4]
                        for n, i in enumerate(valid):
                            mm(ps_se[0:1, cs], ones128_1, probs[i][jq],
                               n == 0, n == len(valid) - 1)
                    se_row = vecs.tile([1, S], f32, tag='se_row', name='se_row')
                    nc.scalar.activation(se_row, ps_se, AF.Copy)
                    se_rec = vecs.tile([1, S], f32, tag='se_rec', name='se_rec')
                    nc.vector.reciprocal_approx_fast(out=se_rec, in_=se_row)
                    se_bf = vecs.tile([1, S], bf16, tag='se_bf', name='se_bf')
                    nc.vector.tensor_copy(se_bf, se_rec)
                    for jq in range(NCH):
                        cs = ts(jq, TCH)
                        psr = ph45ps.tile([P, TCH], f32, tag='at_ps', bufs=2,
                                          name='rec_ps')
                        mm(psr, ones1_128, se_bf[0:1, cs], True, True)
                        recb = ph4.tile([P, TCH], f32, tag='recb', bufs=2, name='recb')
                        nc.vector.tensor_copy(recb, psr)
                        psa = ph45ps.tile([P, TCH], f32, tag='at_ps', bufs=2,
                                          name='at_ps')
                        valid = [i for i in range(NTT) if jq >= i // 4]
                        for n, i in enumerate(valid):
                            mm(psa, v_sb[:, i, ts(j, DV)], probs[i][jq],
                               n == 0, n == len(valid) - 1)
                        a = ph4.tile([P, TCH], bf16, tag='attn_o', name='attn_o')
                        nc.vector.tensor_mul(a, psa, recb)
                        nc.sync.dma_start(out=attn_dram[j][:, cs], in_=a)
                    ag(attn_dram[j], attn_ag[j])

                # o_proj: wave-ordered, accumulate in SBUF fp32
                for w in range(HPC):
                    o_sb = ph4w.tile([P, NC, OC], bf16, tag='o_sb', bufs=2,
                                     name=f'o_sb{w}')
                    nc.sync.dma_start(
                        out=o_sb, in_=o_own[w * NC * P:(w + 1) * NC * P, :]
                        .rearrange('(k p) n -> p k n', p=P))
                    att_w = ph4.tile([P, NC, S], bf16, tag='att_w', bufs=2,
                                     name=f'att_w{w}')
                    for r in range(NC):
                        nc.sync.dma_start(out=att_w[:, r, :],
                                          in_=attn_ag[w][r * DV:(r + 1) * DV, :])
                    for m in range(OC // P):
                        pso = [ph45ps.tile([P, TCH], f32, tag='o_ps', bufs=2,
                                           name=f'o_ps{w}_{m}_{c}')
                               for c in range(NCH)]
                        for k in range(NC):
                            for c in range(NCH):
                                mm(pso[c], o_sb[:, k, ts(m, P)],
                                   att_w[:, k, ts(c, TCH)], k == 0, k == NC - 1)
                        for c in range(NCH):
                            cs = ts(c, TCH)
                            if w == 0:
                                nc.vector.tensor_add(o_acc[:, m, cs], pso[c],
                                                     h_own[:, m, cs])
                            else:
                                nc.vector.tensor_add(o_acc[:, m, cs],
                                                     o_acc[:, m, cs], pso[c])
                        if w == HPC - 1:
                            # h2 chunk m complete: AG with norm-partial row
                            h2b = ph4.tile([P, S], bf16, tag='h2b', name='h2b')
                            nc.vector.tensor_copy(h2b, o_acc[:, m, :])
                            nc.sync.dma_start(out=h2_dram[m][0:P, :], in_=h2b)
                            sq2 = ph4.tile([P, S], bf16, tag='sq2', name='sq2')
                            nc.vector.tensor_mul(sq2, o_acc[:, m, :], o_acc[:, m, :])
                            pph = ph45ps.tile([1, S], f32, tag='se_ps', name='pph')
                            for c in range(NCH):
                                mm(pph[0:1, ts(c, TCH)], ones128_1,
                                   sq2[:, ts(c, TCH)], True, True)
                            h2_row = ph4.tile([1, S], bf16, tag='h2_row', name='h2_row')
                            nc.scalar.activation(h2_row, pph, AF.Copy)
                            nc.sync.dma_start(out=h2_dram[m][P:P + 1, :], in_=h2_row)
                            ag(h2_dram[m], h2_ag[m])

        # ============ MLP: gate/up wave-ordered + chunked m AG =============
        with ExitStack() as mlp_st:
            mlp_sb = mlp_st.enter_context(tc.tile_pool(name='mlp_sb', bufs=1))
            h2T = mlp_sb.tile([P, NDT, S], bf16, name='h2T')
            with tc.tile_pool(name='ph6', bufs=2) as ph6, \
                 tc.tile_pool(name='ph6w', bufs=2) as ph6w, \
                 tc.tile_pool(name='ph6ps', bufs=1, space='PSUM') as ph6ps:
                for w in range(4):
                    for r in range(NC):
                        nc.sync.dma_start(out=h2T[:, w * NC + r, :],
                                          in_=h2_ag[w][r * H2B:r * H2B + P, :])
                rparts = ph6.tile([32, S], bf16, tag='rparts', bufs=1, name='rparts')
                for w in range(4):
                    for r in range(NC):
                        nc.sync.dma_start(
                            out=rparts[w * NC + r:w * NC + r + 1, :],
                            in_=h2_ag[w][r * H2B + P:r * H2B + P + 1, :])
                pr2 = ph6ps.tile([1, S], f32, tag='pr2', bufs=1, name='pr2')
                for c in range(NCH):
                    mm(pr2[0:1, ts(c, TCH)], ones32_1, rparts[:, ts(c, TCH)],
                       True, True)
                r2_row = row_chain(pr2, 1.0 / D, 'r2', ph6)
                r2_b = bcast(r2_row, ph6ps, ph6, 'r2', tag='pr2b')

                for mc in range(NMC):
                    rows = min(P, IC - mc * P)
                    wg = ph6w.tile([P, NDT, rows], bf16, tag='wg', bufs=2, name='wg')
                    nc.sync.dma_start(
                        out=wg, in_=gate_own[:, ds(mc * P, rows)]
                        .rearrange('(k p) n -> p k n', p=P))
                    wu = ph6w.tile([P, NDT, rows], bf16, tag='wu', bufs=2, name='wu')
                    nc.sync.dma_start(
                        out=wu, in_=up_own[:, ds(mc * P, rows)]
                        .rearrange('(k p) n -> p k n', p=P))
                    psg = [ph6ps.tile([rows, TCH], f32, tag='g_ps', bufs=2,
                                      name=f'g_ps{mc}_{c}') for c in range(NCH)]
                    psu = [ph6ps.tile([rows, TCH], f32, tag='u_ps', bufs=2,
                                      name=f'u_ps{mc}_{c}') for c in range(NCH)]
                    for k in range(NDT):
                        for c in range(NCH):
                            mm(psg[c], wg[:, k, :], h2T[:, k, ts(c, TCH)],
                               k == 0, k == NDT - 1)
                    for k in range(NDT):
                        for c in range(NCH):
                            mm(psu[c], wu[:, k, :], h2T[:, k, ts(c, TCH)],
                               k == 0, k == NDT - 1)
                    grp = next(i for i, g in enumerate(MGRP) if g[0] <= mc < g[1])
                    for c in range(NCH):
                        cs = ts(c, TCH)
                        graw = ph6.tile([P, TCH], bf16, tag='graw', name='graw')
                        nc.scalar.activation(graw[0:rows, :], psg[c], AF.Copy)
                        uraw = ph6.tile([P, TCH], bf16, tag='uraw', name='uraw')
                        nc.scalar.activation(uraw[0:rows, :], psu[c], AF.Copy)
                        g2 = ph6.tile([P, TCH], bf16, tag='g2', name='g2')
                        nc.vector.tensor_mul(g2[0:rows, :], graw[0:rows, :],
                                             r2_b[0:rows, cs])
                        nc.scalar.activation(g2[0:rows, :], g2[0:rows, :], AF.Silu)
                        u2 = ph6.tile([P, TCH], bf16, tag='u2', name='u2')
                        nc.vector.tensor_mul(u2[0:rows, :], uraw[0:rows, :],
                                             r2_b[0:rows, cs])
                        mt = ph6.tile([P, TCH], bf16, tag='mt', name='mt')
                        nc.vector.tensor_mul(mt[0:rows, :], g2[0:rows, :],
                                             u2[0:rows, :])
                        nc.sync.dma_start(
                            out=m_dram[grp][ds((mc - MGRP[grp][0]) * P, rows), ts(c, TCH)],
                            in_=mt[0:rows, :])
                    if mc == MGRP[grp][1] - 1:
                        ag(m_dram[grp], m_ag[grp])

        # ============ down_proj + final residual ===========================
        with tc.tile_pool(name='ph7', bufs=4) as ph7, \
             tc.tile_pool(name='ph7ps', bufs=1, space='PSUM') as ph7ps:
            ps_d = [ph7ps.tile([P, S], f32, tag=f'd_ps{m}', name=f'd_ps{m}')
                    for m in range(OC // P)]
            G7 = 2
            kglob = 0
            woff = 0
            nkt_tot = INTER // P
            for grp, (_, _, grows) in enumerate(MGRP):
                ntiles = NC * grows // P
                for g in range(ntiles // G7):
                    mk = ph7.tile([P, G7, S], bf16, tag='mk', name='mk')
                    nc.sync.dma_start(
                        out=mk, in_=m_ag[grp][g * G7 * P:(g + 1) * G7 * P, :]
                        .rearrange('(k p) s -> p k s', p=P))
                    dw = ph7.tile([P, G7, OC], bf16, tag='dw', name='dw')
                    nc.sync.dma_start(
                        out=dw, in_=down_own[woff + g * G7 * P:woff + (g + 1) * G7 * P, :]
                        .rearrange('(k p) n -> p k n', p=P))
                    for kk in range(G7):
                        k = kglob + g * G7 + kk
                        for m in range(OC // P):
                            for c in range(NCH):
                                mm(ps_d[m][:, ts(c, TCH)], dw[:, kk, ts(m, P)],
                                   mk[:, kk, ts(c, TCH)], k == 0, k == nkt_tot - 1)
                kglob += ntiles
                woff += ntiles * P
            for m in range(OC // P):
                o = ph7.tile([P, S], f32, tag='o_out', name='o_out')
                nc.vector.tensor_add(o, ps_d[m], o_acc[:, m, :])
                nc.sync.dma_start(out=out[ts(m, P), :], in_=o)

    nc.compile()
    return nc


def _prep_inputs(inputs):
    """Host-side sharding: returns list of 8 per-core input dicts."""
    h = np.ascontiguousarray(np.asarray(inputs['hidden_states'], np.float32))
    hT = np.ascontiguousarray(h.T)
    cosT = np.ascontiguousarray(np.asarray(inputs['cos'], np.float32).T)
    sinT = np.ascontiguousarray(np.asarray(inputs['sin'], np.float32).T)
    q_a_w = np.asarray(inputs['q_a_w'], np.float32)
    q_b_w = np.asarray(inputs['q_b_w'], np.float32)
    kv_a_w = np.asarray(inputs['kv_a_w'], np.float32)
    kv_b_w = np.asarray(inputs['kv_b_w'], np.float32)
    o_w = np.asarray(inputs['o_w'], np.float32)
    gate_w = np.asarray(inputs['gate_w'], np.float32)
    up_w = np.asarray(inputs['up_w'], np.float32)
    down_w = np.asarray(inputs['down_w'], np.float32)

    pidx = np.arange(P)[:, None]
    cidx = np.arange(TCH)[None, :]
    masks = np.stack([(cidx - pidx >= P * k) for k in range(4)]).astype(BF16)

    cosT2 = np.ascontiguousarray(np.vstack([cosT, cosT]))
    sinT2 = np.ascontiguousarray(np.vstack([sinT, sinT]))
    R = np.zeros((DR, DR), np.float32)
    R[np.arange(DR // 2), np.arange(DR // 2) + DR // 2] = -1.0
    R[np.arange(DR // 2) + DR // 2, np.arange(DR // 2)] = 1.0
    R2 = np.zeros((P, P), np.float32)
    R2[:DR, :DR] = R
    R2[DR:, DR:] = R
    rot2T = np.ascontiguousarray(R2.T)

    # gate/up rows reordered to h2-AG wave order: (chunk w, core r) -> rows
    # r*512 + w*128 .. +128
    gu_order = np.concatenate(
        [np.arange(r * OC + w * P, r * OC + (w + 1) * P)
         for w in range(4) for r in range(NC)])
    # o rows reordered to attention-AG wave order: (head-slot w, core r)
    o_order = np.concatenate(
        [np.arange((r * HPC + w) * DV, (r * HPC + w + 1) * DV)
         for w in range(HPC) for r in range(NC)])
    # down rows reordered to m-AG group order
    m_row_order = np.concatenate(
        [np.arange(lo * P, lo * P + grows) + r * IC
         for (lo, hi, grows) in MGRP for r in range(NC)])

    in_maps = []
    for r in range(NC):
        heads = range(r * HPC, (r + 1) * HPC)
        qb_cols = np.concatenate(
            [q_b_w[:, hh * (DN + DR):hh * (DN + DR) + DN] for hh in heads] +
            [q_b_w[:, hh * (DN + DR) + DN:(hh + 1) * (DN + DR)] for hh in heads],
            axis=1)
        kvb_cols = np.concatenate(
            [kv_b_w[:, hh * (DN + DV):hh * (DN + DV) + DN] for hh in heads] +
            [kv_b_w[:, hh * (DN + DV) + DN:(hh + 1) * (DN + DV)] for hh in heads],
            axis=1)
        aw_cols = np.concatenate(
            [kv_a_w[:, r * KVAC:(r + 1) * KVAC],
             np.zeros((D, APAD), np.float32),
             q_a_w[:, r * QAC:(r + 1) * QAC]], axis=1)
        kvmask = (np.arange(r * KVAC, (r + 1) * KVAC) < KVLORA
                  ).astype(BF16)[:, None]
        in_maps.append({
            'hT': hT.astype(BF16),
            'h_ownD': np.ascontiguousarray(hT[r * OC:(r + 1) * OC]),
            'aw': np.ascontiguousarray(aw_cols).astype(BF16),
            'kvmask': kvmask,
            'qb_own': np.ascontiguousarray(qb_cols).astype(BF16),
            'kvb_own': np.ascontiguousarray(kvb_cols).astype(BF16),
            'o_own': np.ascontiguousarray(
                o_w[o_order, r * OC:(r + 1) * OC]).astype(BF16),
            'gate_own': np.ascontiguousarray(
                gate_w[gu_order, r * IC:(r + 1) * IC]).astype(BF16),
            'up_own': np.ascontiguousarray(
                up_w[gu_order, r * IC:(r + 1) * IC]).astype(BF16),
            'down_own': np.ascontiguousarray(
                down_w[m_row_order, r * OC:(r + 1) * OC]).astype(BF16),
            'cosT2': cosT2,
            'sinT2': sinT2,
            'rot2T': rot2T.astype(BF16),
            'masks': masks,
        })
    return in_maps


def kernel(**inputs) -> np.ndarray:
    if 'nc' not in _CACHE:
        _CACHE['nc'] = _build()
    nc = _CACHE['nc']
    from concourse.bass_utils import run_bass_kernel_spmd
    in_maps = _prep_inputs(inputs)
    res = run_bass_kernel_spmd(nc, in_maps, core_ids=list(range(NC)))
    outT = np.concatenate([res.results[r]['out'] for r in range(NC)], axis=0)
    return np.ascontiguousarray(outT.T)
